# revision 1
# baseline (speedup 1.0000x reference)
"""CTRNN cell (6 Euler unfolds) on 8 Trainium2 NeuronCores.

Math (per unfold, 6x):
    f     = tanh([x, s] @ W + b)
    s_new = s + 0.1 * (-s + f)  = 0.9*s + 0.1*f

Strategy:
  - Data-parallel over batch: B=8192 -> 1024 rows/core, no cross-core
    communication. Host does the cheap numpy transposes/packing.
  - Everything kept TRANSPOSED on-chip (feature dim on SBUF partitions,
    batch on the free dim) so state feeds the tensor engine as the moving
    operand and W slices are directly the stationary lhsT.
  - pre = x @ W_top is computed once. Per-unfold matmuls run in *delta*
    form: one PSUM accumulator per output m-tile holds pre + s_k @ W_bot
    across all unfolds, updated with psum += (f_k - s_k) @ (0.1*W_bot).
    This is the 7-logical-matmul FLOP floor and PSUM never restarts.
  - All matmuls in float32r (fp32-precision inputs at bf16 rate for free
    dim >= 512). Inputs are DMA'd directly into f32r-typed tiles (walrus
    accepts a DMACopy with f32r output dtype as the required rounding
    producer; verified bit-identical to an explicit DVE cast on HW).
    The state is kept in plain f32 (so the per-unfold update never rounds
    it to f32r); a one-time f32r copy feeds the init matmul.
  - x, s and W all arrive host-packed as (128, k*1024) layouts so every
    DMA has >=4KB-contiguous per-partition runs - the DMA queues are
    descriptor-rate bound (4KB is the max HW descriptor; 2KB rows halve
    bandwidth). Input is spread over the SWDGE path and both HWDGE rings.
  - PSUM per m-tile is one (128,1024) span (2 banks); matmuls write
    512-wide halves, tanh reads the full span (amortizes ACT op overhead).
  - bias is folded into the tanh activation's per-partition bias operand.
  - A junk-matmul warm-up keeps the PE activity monitor from throttling
    the clock to 1.2 GHz during the input-load phase.
  - Steady state is vector-engine paced (2 fused scalar_tensor_tensor
    passes per m-tile per unfold: tmp = f - s, then s += 0.1*tmp).
"""

import numpy as np

UNFOLDS = 6
DT = 0.1
B, D, N = 8192, 512, 512
NCORES = 8
BC = B // NCORES          # batch rows per core
CHUNK = 512               # matmul moving-operand free dim (PSUM bank)
NCHUNKS = BC // CHUNK     # 2
P = 128
KT_X = D // P             # k-tiles of W_top
KT_S = N // P             # k-tiles of W_bot
MT = N // P               # m-tiles of the output dim

_compiled_nc = None


def _build_nc():
    import concourse.bass as bass  # noqa: F401
    import concourse.bacc as bacc
    import concourse.tile as tile
    from concourse import mybir

    f32 = mybir.dt.float32
    f32r = mybir.dt.float32r
    bf16 = mybir.dt.bfloat16
    MULT = mybir.AluOpType.mult
    ADD = mybir.AluOpType.add
    TANH = mybir.ActivationFunctionType.Tanh

    nc = bacc.Bacc("TRN2", target_bir_lowering=False, debug=False)

    xP = nc.dram_tensor("xP", [P, D * BC // P], f32r, kind="ExternalInput").ap()
    sP = nc.dram_tensor("sP", [P, N * BC // P], f32, kind="ExternalInput").ap()
    Wp_d = nc.dram_tensor("Wp", [P, (D + N) * N // P], f32r,
                          kind="ExternalInput").ap()
    bias = nc.dram_tensor("bias", [N], f32, kind="ExternalInput").ap()
    outT = nc.dram_tensor("outT", [N, BC], f32, kind="ExternalOutput").ap()

    with tile.TileContext(nc) as tc:
        with (
            tc.tile_pool(name="weights", bufs=1) as wpool,
            tc.tile_pool(name="dmain", bufs=3) as dmain,
            tc.tile_pool(name="data", bufs=1) as data,
            tc.tile_pool(name="tmp", bufs=2) as tmpp,
            tc.tile_pool(name="fpool", bufs=3) as fpool,
            tc.tile_pool(name="psum", bufs=1, space="PSUM") as psump,
        ):
            # ---- input DMAs (all f32r-direct, no rounding casts) -----------
            # walrus accepts DMACopy with f32r output as the rounding
            # producer for f32r matmuls (verified on HW: identical result to
            # an explicit DVE cast). W arrives host-packed as (128, 4096) so
            # every DMA has 4KB-contiguous per-partition runs (the DMA queues
            # are descriptor-rate-bound: 2KB rows halve the bandwidth).
            # Load is balanced across SWDGE (~200 GB/s) and the two HWDGE
            # rings; everything lands by ~HBM-bound time.
            # HAM warm-up part 1: memset a junk tile first thing on the
            # gpsimd queue (before its DMA issues) so the warm-up matmuls can
            # start immediately.
            junk = wpool.tile([P, N], bf16, tag="junk", name="junk")
            nc.gpsimd.memset(junk[:], 0)

            NPAIR = (KT_X + KT_S) // 2
            # x and s are host-packed like W so every DMA has >=8KB-
            # contiguous per-partition runs (DMA queues are descriptor-rate
            # bound). x on the sync ring, s on SWDGE, W on the scalar ring;
            # two half-DMAs per tensor so the first k-tiles land early.
            wp = []
            for q in range(NPAIR):
                wd = wpool.tile([P, 2 * N], f32r, tag=f"wp{q}", name=f"wp{q}")
                eng = nc.gpsimd if q == NPAIR - 1 else nc.scalar
                eng.dma_start(wd[:], Wp_d[:, q * 2 * N:(q + 1) * 2 * N])
                wp.append(wd)

            HALF = D * BC // P // 2
            x_mega = data.tile([P, 2 * HALF], f32r, tag="xm", name="x_mega")
            nc.sync.dma_start(x_mega[:, 0:HALF], xP[:, 0:HALF])
            nc.sync.dma_start(x_mega[:, HALF:2 * HALF], xP[:, HALF:2 * HALF])
            x_sb = [x_mega[:, j * BC:(j + 1) * BC] for j in range(KT_X)]

            s_mega = data.tile([P, 2 * HALF], f32, tag="sm", name="s_mega")
            nc.gpsimd.dma_start(s_mega[:, 0:HALF], sP[:, 0:HALF])
            nc.gpsimd.dma_start(s_mega[:, HALF:2 * HALF], sP[:, HALF:2 * HALF])
            s_sb = [s_mega[:, j * BC:(j + 1) * BC] for j in range(KT_S)]

            s_r = []
            for j in range(KT_S):
                tr = data.tile([P, BC], f32r, tag=f"sr{j}", name=f"sr{j}")
                nc.vector.tensor_copy(tr[:], s_sb[j])
                s_r.append(tr)
            bias_sb = wpool.tile([P, MT], f32, tag="bias", name="bias_sb")
            nc.gpsimd.dma_start(bias_sb[:], bias.rearrange("(m p) -> p m", p=P))

            # the only casts left: 0.1*W_bot in bf16 for the delta matmuls
            wbp01h = []
            for q in range(NPAIR // 2):
                w = wpool.tile([P, 2 * N], f32r, tag=f"wbph{q}",
                               name=f"wbp01h_{q}")
                nc.scalar.mul(w[:], wp[NPAIR // 2 + q][:], DT)
                wbp01h.append(w)

            def pair_slices(pairs):
                out = []
                for w in pairs:
                    out.append(w[:, 0:N])
                    out.append(w[:, N:2 * N])
                return out

            wt = pair_slices(wp[:NPAIR // 2])       # W_top f32r k-slices
            wbot = pair_slices(wp[NPAIR // 2:])     # W_bot f32r k-slices
            wb01h = pair_slices(wbp01h)             # 0.1*W_bot bf16 k-slices

            # ---- persistent PSUM accumulators: pre + s_k @ W_bot ----------
            # one (128, 1024) span per m-tile = 2 banks; matmuls address
            # 512-wide halves, ACT reads the whole span.
            ps = [psump.tile([P, BC], f32, tag=f"ps{m}", name=f"ps{m}")
                  for m in range(MT)]

            # HAM warm-up part 2: junk matmuls keep the PE busy while the
            # inputs stream in, so real matmuls run at 2.4 GHz from the start
            # (the activity monitor needs ~3.4us of sustained work to
            # unthrottle). Overwritten by the first start=True matmul per bank.
            for r in range(20):
                nc.tensor.matmul(
                    ps[r % MT][:, 0:CHUNK],
                    lhsT=junk[:, 0:P], rhs=junk[:, 0:CHUNK],
                    start=True, stop=True, skip_group_check=True,
                )

            def mm_round(weights, rhs_tiles, start, stop, m_outer=False):
                nkt = len(rhs_tiles)
                order = (
                    [(j, m) for m in range(MT) for j in range(nkt)]
                    if m_outer else
                    [(j, m) for j in range(nkt) for m in range(MT)]
                )
                for j, m in order:
                    for c in range(NCHUNKS):
                        nc.tensor.matmul(
                            ps[m][:, c * CHUNK:(c + 1) * CHUNK],
                            lhsT=weights[j][:, m * P:(m + 1) * P],
                            rhs=rhs_tiles[j][:, c * CHUNK:(c + 1) * CHUNK],
                            start=(start and j == 0),
                            stop=(stop and j == nkt - 1),
                            skip_group_check=True,
                        )

            # init: psum = x @ W_top + s0 @ W_bot
            mm_round(wt, x_sb, start=True, stop=False)
            mm_round(wb01h, s_r, start=False, stop=False)

            # ---- unfolds ---------------------------------------------------
            # state kept scaled: v = 10*s. tmp = f - 0.1*v (== f - s) feeds
            # the delta matmuls; the state update becomes the plain add
            # v += tmp, which runs on the otherwise-idle GpSimd engine and
            # frees half of the vector-engine work per unfold.
            for k in range(UNFOLDS):
                last = k == UNFOLDS - 1
                tmp_t = [tmpp.tile([P, BC], f32r, tag=f"tmp{j}",
                                   name=f"tmp{k}_{j}")
                         for j in range(MT)]
                f_t = [fpool.tile([P, BC], f32, tag=f"f{m}", name=f"f{k}_{m}",
                                  bufs=2)
                       for m in range(MT)]
                if not last:
                    for m in range(MT):
                        # f = tanh(psum + bias), full (128,1024) span
                        nc.scalar.activation(
                            f_t[m][:], ps[m][:], TANH,
                            bias=bias_sb[:, m:m + 1], scale=1.0,
                        )
                        # tmp = f - 0.1*v (f32r out, feeds the delta matmuls)
                        nc.vector.scalar_tensor_tensor(
                            tmp_t[m][:], s_sb[m], -DT, f_t[m][:],
                            op0=MULT, op1=ADD,
                        )
                    # psum += tmp @ (0.1*W_bot)
                    mm_round(wb01h, tmp_t, start=False,
                             stop=(k == UNFOLDS - 2))
                    # v += tmp (plain add, lazy: emitted after the matmuls)
                    for m in range(MT):
                        nc.vector.tensor_tensor(
                            s_sb[m], s_sb[m], tmp_t[m][:], ADD,
                        )
                else:
                    # final unfold: s_out = 0.1*(0.9*v + f) = 0.9*s + 0.1*f;
                    # the descale is a fast single-src tensor_scalar
                    for m in range(MT):
                        nc.scalar.activation(
                            f_t[m][:], ps[m][:], TANH,
                            bias=bias_sb[:, m:m + 1], scale=1.0,
                        )
                        nc.vector.scalar_tensor_tensor(
                            f_t[m][:], s_sb[m], 0.9, f_t[m][:],
                            op0=MULT, op1=ADD,
                        )
                        nc.vector.tensor_scalar_mul(
                            s_sb[m].bitcast(f32), f_t[m][:], DT)
                        out_eng = (nc.sync, nc.scalar, nc.gpsimd,
                                   nc.sync)[m]
                        out_eng.dma_start(outT[m * P:(m + 1) * P, :],
                                          s_sb[m].bitcast(f32))

    nc.compile()
    return nc


def _get_nc():
    global _compiled_nc
    if _compiled_nc is None:
        _compiled_nc = _build_nc()
    return _compiled_nc


def make_in_maps(x, s, W, b):
    """Shard + pack host-side: x/s/W packed to (128, k*1024) with k-tiles
    side by side so per-partition runs are >=8KB contiguous."""
    xT = np.ascontiguousarray(x.T)   # (D, B)
    sTf = np.ascontiguousarray(s.T)  # (N, B)
    Wp = np.ascontiguousarray(
        W.reshape(4, 2, P, N).transpose(2, 0, 1, 3).reshape(P, -1))
    in_maps = []
    for c in range(NCORES):
        sl = slice(c * BC, (c + 1) * BC)
        xs = xT[:, sl].reshape(KT_X, P, BC).transpose(1, 0, 2).reshape(P, -1)
        ss = (10.0 * sTf[:, sl]).reshape(KT_S, P, BC).transpose(1, 0, 2).reshape(P, -1)
        in_maps.append({
            "xP": np.ascontiguousarray(xs),
            "sP": np.ascontiguousarray(ss),
            "Wp": Wp,
            "bias": b,
        })
    return in_maps


def kernel(**inputs):
    from concourse.bass_utils import run_bass_kernel_spmd

    x = np.asarray(inputs["inputs"], dtype=np.float32)
    s = np.asarray(inputs["state"], dtype=np.float32)
    W = np.ascontiguousarray(np.asarray(inputs["W"], dtype=np.float32))
    b = np.ascontiguousarray(np.asarray(inputs["bias"], dtype=np.float32))

    in_maps = make_in_maps(x, s, W, b)
    nc = _get_nc()
    res = run_bass_kernel_spmd(nc, in_maps, list(range(NCORES))).results
    outT = np.concatenate([res[c]["outT"] for c in range(NCORES)], axis=1)
    out = np.ascontiguousarray(outT.T).astype(np.float32)
    return (out, out)



# revision 4
# speedup vs baseline: 1.2670x; 1.2670x over previous
"""CTRNN cell (6 Euler unfolds) on 8 Trainium2 NeuronCores.

Math (per unfold, 6x):
    f     = tanh([x, s] @ W + b)
    s_new = s + 0.1 * (-s + f)  = 0.9*s + 0.1*f

v2 strategy (over the f32r v1 baseline):
  - Data-parallel over batch: B=8192 -> 1024 rows/core, no cross-core
    communication. Host does cheap numpy transposes/packing/casts.
  - Everything transposed on-chip (features on partitions, batch on free).
  - Delta form: psum holds alpha*(x@Wt + s_k@Wb); per unfold
    psum += tmp_k @ (alpha*0.1*Wb) with tmp_k = f_k - s_k. 7-matmul floor.
  - fp16 init matmuls (x@alphaWt + s0@alphaWb): FWL weight loads (fp32
    disables FWL), half the DMA bytes of f32r, PE at 1 col/cycle.
  - fp8e4 DoubleRow delta matmuls (0.5 cyc/row): weights are
    Wd = alpha*0.1*Wb with alpha=64 so Wd (rms ~0.23) sits in e4m3's
    normal range (0.1*Wb alone would be entirely subnormal).
    The tanh activation descales via its scale operand (1/alpha).
  - State kept in v-form v_k = s_k/0.9^k so every state update is a single
    scalar_tensor_tensor: v += (0.1/0.9^(k+1))*f. tmp_k = f_k - 0.9^k*v_k
    is one DVE stt with fp8 output feeding the DoubleRow matmuls directly.
    v-updates split across gpsimd+vector to keep DVE under the PE round.
  - fp16 tensors everywhere (DVE 2-byte perf modes, half DMA); output
    shipped fp16 and upcast to f32 on host (0.05% quantization, tolerance
    is 2e-2 and baseline measured 1.3e-4).
  - Junk-matmul warm-up keeps the PE activity monitor from throttling
    during the input-load phase.
"""

import numpy as np

UNFOLDS = 6
DT = 0.1
B, D, N = 8192, 512, 512
NCORES = 8
BC = B // NCORES          # batch rows per core
CHUNK = 512               # matmul moving free dim (one PSUM bank of f32)
NCH = BC // CHUNK         # 2
P = 128
KT_X = D // P             # 4 k-tiles of W_top
KT_S = N // P             # 4 k-tiles of W_bot
MT = N // P               # 4 m-tiles of the output dim
ALPHA = 64.0              # psum domain scale (keeps fp8 Wd in normal range)

_compiled_nc = None


def _build_nc():
    import concourse.bass as bass  # noqa: F401
    import concourse.bacc as bacc
    import concourse.tile as tile
    from concourse import mybir

    f32 = mybir.dt.float32
    f16 = mybir.dt.float16
    f8 = mybir.dt.float8e4
    MULT = mybir.AluOpType.mult
    ADD = mybir.AluOpType.add
    TANH = mybir.ActivationFunctionType.Tanh
    DR = mybir.MatmulPerfMode.DoubleRow

    nc = bacc.Bacc("TRN2", target_bir_lowering=False, debug=False)

    xP = nc.dram_tensor("xP", [P, D * BC // P], f16, kind="ExternalInput").ap()
    sP = nc.dram_tensor("sP", [P, N * BC // P], f16, kind="ExternalInput").ap()
    Wp_d = nc.dram_tensor("Wp", [P, (D + N) * N // P], f16,
                          kind="ExternalInput").ap()
    bias = nc.dram_tensor("bias", [N], f32, kind="ExternalInput").ap()
    outT = nc.dram_tensor("outT", [N, BC], f16, kind="ExternalOutput").ap()

    with tile.TileContext(nc) as tc:
        with (
            tc.tile_pool(name="weights", bufs=1) as wpool,
            tc.tile_pool(name="data", bufs=1) as data,
            tc.tile_pool(name="tmp", bufs=2) as tmpp,
            tc.tile_pool(name="fpool", bufs=2) as fpool,
            tc.tile_pool(name="opool", bufs=1) as opool,
            tc.tile_pool(name="psum", bufs=1, space="PSUM") as psump,
        ):
            # ---- input DMAs ------------------------------------------------
            # junk memset first on gpsimd so warm-up matmuls start instantly.
            junk = wpool.tile([P, CHUNK], f16, tag="junk", name="junk")
            nc.gpsimd.memset(junk[:], 0)

            # W on the scalar ring (two 4KB/partition halves: Wt then Wb)
            HALF_W = (D + N) * N // P // 2
            w_sb = data.tile([P, 2 * HALF_W], f16, tag="w", name="w_sb")
            nc.scalar.dma_start(w_sb[:, 0:HALF_W], Wp_d[:, 0:HALF_W])
            nc.scalar.dma_start(w_sb[:, HALF_W:2 * HALF_W],
                                Wp_d[:, HALF_W:2 * HALF_W])

            # x on the sync ring, s on gpsimd; 4KB/partition halves.
            HALF = D * BC // P // 2
            x_sb = data.tile([P, 2 * HALF], f16, tag="x", name="x_sb")
            nc.sync.dma_start(x_sb[:, 0:HALF], xP[:, 0:HALF])
            nc.sync.dma_start(x_sb[:, HALF:2 * HALF], xP[:, HALF:2 * HALF])

            v_sb = data.tile([P, 2 * HALF], f16, tag="v", name="v_sb")
            nc.gpsimd.dma_start(v_sb[:, 0:HALF], sP[:, 0:HALF])
            nc.gpsimd.dma_start(v_sb[:, HALF:2 * HALF], sP[:, HALF:2 * HALF])

            bias_sb = wpool.tile([P, MT], f32, tag="bias", name="bias_sb")
            nc.gpsimd.dma_start(bias_sb[:], bias.rearrange("(m p) -> p m", p=P))

            # fp8 DoubleRow delta weights: Wd = 0.1 * (alpha*Wb), packed as
            # [128, kgroup(2), 512] per k-pair q (features 256q+128g+p).
            wd = []
            for q in range(KT_S // 2):
                w8 = wpool.tile([P, 2, N], f8, tag=f"wd{q}", name=f"wd{q}")
                for g in range(2):
                    j = KT_X + 2 * q + g
                    nc.scalar.mul(w8[:, g:g + 1, :],
                                  w_sb[:, j * N:(j + 1) * N], DT)
                wd.append(w8)

            # ---- persistent PSUM accumulators ------------------------------
            ps = [psump.tile([P, BC], f32, tag=f"ps{m}", name=f"ps{m}")
                  for m in range(MT)]

            # HAM warm-up: junk matmuls keep the PE busy while inputs stream
            # in (activity monitor needs ~3.4us sustained work to unthrottle).
            for r in range(10):
                nc.tensor.matmul(
                    ps[r % MT][:, 0:CHUNK],
                    lhsT=junk[:, 0:P], rhs=junk[:, 0:CHUNK],
                    start=True, stop=True, skip_group_check=True,
                )

            def wslice(j, m):
                return w_sb[:, j * N + m * P: j * N + (m + 1) * P]

            # init: psum = x @ (alpha*Wt) + s0 @ (alpha*Wb), fp16
            for part, rhs_sb, kt0 in ((0, x_sb, 0), (1, v_sb, KT_X)):
                for j in range(4):
                    for m in range(MT):
                        for c in range(NCH):
                            nc.tensor.matmul(
                                ps[m][:, c * CHUNK:(c + 1) * CHUNK],
                                lhsT=wslice(kt0 + j, m),
                                rhs=rhs_sb[:, j * BC + c * CHUNK:
                                           j * BC + (c + 1) * CHUNK],
                                start=(part == 0 and j == 0),
                                stop=False,
                                skip_group_check=True,
                            )

            # ---- unfolds ---------------------------------------------------
            for k in range(UNFOLDS):
                last = k == UNFOLDS - 1
                sk = 0.9 ** k            # s_k = 0.9^k * v_k
                ck = DT / 0.9 ** (k + 1)  # v += ck * f
                f_t = [fpool.tile([P, BC], f16, tag=f"f{m}", name=f"f{k}_{m}")
                       for m in range(MT)]
                for m in range(MT):
                    # f = tanh(psum/alpha + bias), full (128,1024) span
                    nc.scalar.activation(
                        f_t[m][:], ps[m][:], TANH,
                        bias=bias_sb[:, m:m + 1], scale=1.0 / ALPHA,
                    )
                if not last:
                    tmp_t = [tmpp.tile([P, 2, BC], f8, tag=f"tmp{q}",
                                       name=f"tmp{k}_{q}")
                             for q in range(MT // 2)]
                    for m in range(MT):
                        q, g = divmod(m, 2)
                        # tmp = f - 0.9^k * v  (fp8 out, feeds DoubleRow)
                        nc.vector.scalar_tensor_tensor(
                            tmp_t[q][:, g:g + 1, :],
                            v_sb[:, m * BC:(m + 1) * BC], -sk, f_t[m][:],
                            op0=MULT, op1=ADD,
                        )
                    # psum += tmp @ Wd   (fp8e4 DoubleRow, 0.5 cyc/row)
                    for q in range(MT // 2):
                        for m in range(MT):
                            for c in range(NCH):
                                nc.tensor.matmul(
                                    ps[m][:, c * CHUNK:(c + 1) * CHUNK],
                                    lhsT=wd[q][:, :, m * P:(m + 1) * P],
                                    rhs=tmp_t[q][:, :, c * CHUNK:(c + 1) * CHUNK],
                                    start=False,
                                    stop=(k == UNFOLDS - 2 and q == MT // 2 - 1),
                                    perf_mode=DR,
                                    skip_group_check=True,
                                )
                    # v += ck * f  (gpsimd can't run stt in this toolchain)
                    for m in range(MT):
                        eng = nc.vector
                        eng.scalar_tensor_tensor(
                            v_sb[:, m * BC:(m + 1) * BC], f_t[m][:], ck,
                            v_sb[:, m * BC:(m + 1) * BC], op0=MULT, op1=ADD,
                        )
                else:
                    SCALE6 = 0.9 ** UNFOLDS
                    o_t = [opool.tile([P, BC], f16, tag=f"o{m}",
                                      name=f"o{m}") for m in range(MT)]
                    for m in range(MT):
                        nc.vector.scalar_tensor_tensor(
                            v_sb[:, m * BC:(m + 1) * BC], f_t[m][:], ck,
                            v_sb[:, m * BC:(m + 1) * BC], op0=MULT, op1=ADD,
                        )
                        # s_6 = 0.9^6 * v_6
                        nc.vector.tensor_scalar_mul(
                            o_t[m][:], v_sb[:, m * BC:(m + 1) * BC], SCALE6)
                        out_eng = (nc.sync, nc.scalar, nc.gpsimd, nc.sync)[m]
                        out_eng.dma_start(outT[m * P:(m + 1) * P, :], o_t[m][:])

    nc.compile()
    return nc


def _get_nc():
    global _compiled_nc
    if _compiled_nc is None:
        _compiled_nc = _build_nc()
    return _compiled_nc


def make_in_maps(x, s, W, b):
    """Shard + pack host-side: x/s transposed to (128, k*1024) fp16 with
    k-tiles side by side (4KB-contiguous per-partition runs); W scaled by
    ALPHA and packed to (128, 8*512) fp16 k-tile-major."""
    xT = np.ascontiguousarray(x.T)   # (D, B)
    sT = np.ascontiguousarray(s.T)   # (N, B)
    Wp = np.ascontiguousarray(
        (ALPHA * W).reshape(8, P, N).transpose(1, 0, 2).reshape(P, -1)
    ).astype(np.float16)
    in_maps = []
    for c in range(NCORES):
        sl = slice(c * BC, (c + 1) * BC)
        xs = xT[:, sl].reshape(KT_X, P, BC).transpose(1, 0, 2).reshape(P, -1)
        ss = sT[:, sl].reshape(KT_S, P, BC).transpose(1, 0, 2).reshape(P, -1)
        in_maps.append({
            "xP": np.ascontiguousarray(xs).astype(np.float16),
            "sP": np.ascontiguousarray(ss).astype(np.float16),
            "Wp": Wp,
            "bias": b,
        })
    return in_maps


def kernel(**inputs):
    from concourse.bass_utils import run_bass_kernel_spmd

    x = np.asarray(inputs["inputs"], dtype=np.float32)
    s = np.asarray(inputs["state"], dtype=np.float32)
    W = np.ascontiguousarray(np.asarray(inputs["W"], dtype=np.float32))
    b = np.ascontiguousarray(np.asarray(inputs["bias"], dtype=np.float32))

    in_maps = make_in_maps(x, s, W, b)
    nc = _get_nc()
    res = run_bass_kernel_spmd(nc, in_maps, list(range(NCORES))).results
    outT = np.concatenate([res[c]["outT"] for c in range(NCORES)], axis=1)
    out = np.ascontiguousarray(outT.T).astype(np.float32)
    return (out, out)


# revision 6
# speedup vs baseline: 1.2919x; 1.0196x over previous
"""CTRNN cell (6 Euler unfolds) on 8 Trainium2 NeuronCores.

Math (per unfold, 6x):
    f     = tanh([x, s] @ W + b)
    s_new = s + 0.1 * (f - s)

v3 strategy (measured-rate driven):
  - Data-parallel over batch: B=8192 -> 1024 rows/core, no cross-core comm.
  - All tensors fp16. Measured: fp16 MMs hit the 216ns/512-col streaming
    rate with FWL weight loads fully hidden; fp8 DoubleRow measured 1.0x
    (dropped). DVE rates: tensor_tensor 684ns, tensor_scalar 427ns,
    scalar_tensor_tensor 1218ns (avoided) per (128,1024) tile.
  - Delta form: psum holds x@Wt + s_k@Wb; per unfold
    psum += tmp_k @ (0.1*Wb), tmp_k = f_k - s_k.  7-matmul FLOP floor.
  - Per round per m-tile:
      tmp = f - s        DVE tensor_tensor subtract (684ns, short path)
      u   = 0.1*tmp      DVE tensor_scalar_mul (427ns)
      s  += u            gpsimd DMA-accumulate (rides idle DMA engines)
    DVE/round 4.4us, ACT 4.45us, both under the 6.9us PE round.
  - Delta rounds j-outer so ps[0] releases at 76% of the round; next
    round's first MMs need only tmp[0] whose path is ACT+TT = 1.8us.
  - Output = final s update, shipped fp16, upcast to f32 on host
    (tolerance 2e-2; fp16 quantization ~5e-4).
"""

import numpy as np

UNFOLDS = 6
DT = 0.1
B, D, N = 8192, 512, 512
NCORES = 8
BC = B // NCORES          # batch rows per core
CHUNK = 512               # matmul moving free dim (one PSUM bank of f32)
NCH = BC // CHUNK         # 2
P = 128
KT_X = D // P             # 4 k-tiles of W_top
KT_S = N // P             # 4 k-tiles of W_bot
MT = N // P               # 4 m-tiles of the output dim

_compiled_nc = None


def _build_nc():
    import concourse.bass as bass  # noqa: F401
    import concourse.bacc as bacc
    import concourse.tile as tile
    from concourse import mybir

    f32 = mybir.dt.float32
    f16 = mybir.dt.float16
    ADD = mybir.AluOpType.add
    SUB = mybir.AluOpType.subtract
    TANH = mybir.ActivationFunctionType.Tanh

    nc = bacc.Bacc("TRN2", target_bir_lowering=False, debug=False)

    xP = nc.dram_tensor("xP", [P, D * BC // P], f16, kind="ExternalInput").ap()
    sP = nc.dram_tensor("sP", [P, N * BC // P], f16, kind="ExternalInput").ap()
    Wp_d = nc.dram_tensor("Wp", [P, (D + N) * N // P], f16,
                          kind="ExternalInput").ap()
    bias = nc.dram_tensor("bias", [N], f32, kind="ExternalInput").ap()
    outT = nc.dram_tensor("outT", [N, BC], f16, kind="ExternalOutput").ap()

    with tile.TileContext(nc) as tc:
        with (
            tc.tile_pool(name="weights", bufs=1) as wpool,
            tc.tile_pool(name="data", bufs=1) as data,
            tc.tile_pool(name="tmp", bufs=2) as tmpp,
            tc.tile_pool(name="fpool", bufs=2) as fpool,
            tc.tile_pool(name="hpool", bufs=2) as hpool,
            tc.tile_pool(name="opool", bufs=1) as opool,
            tc.tile_pool(name="psum", bufs=1, space="PSUM") as psump,
        ):
            # ---- input DMAs ------------------------------------------------
            junk = wpool.tile([P, CHUNK], f16, tag="junk", name="junk")
            nc.gpsimd.memset(junk[:], 0)

            # first k-tiles of W and x land first (fine-grained leading DMAs)
            w_sb = data.tile([P, (D + N) * N // P], f16, tag="w", name="w_sb")
            nc.scalar.dma_start(w_sb[:, 0:N], Wp_d[:, 0:N])
            nc.scalar.dma_start(w_sb[:, N:4 * N], Wp_d[:, N:4 * N])
            nc.scalar.dma_start(w_sb[:, 4 * N:8 * N], Wp_d[:, 4 * N:8 * N])

            x_sb = data.tile([P, D * BC // P], f16, tag="x", name="x_sb")
            nc.sync.dma_start(x_sb[:, 0:BC], xP[:, 0:BC])
            nc.sync.dma_start(x_sb[:, BC:2 * BC], xP[:, BC:2 * BC])
            nc.sync.dma_start(x_sb[:, 2 * BC:4 * BC], xP[:, 2 * BC:4 * BC])

            s_sb = data.tile([P, N * BC // P], f16, tag="s", name="s_sb")
            nc.gpsimd.dma_start(s_sb[:, 0:2 * BC], sP[:, 0:2 * BC])
            nc.gpsimd.dma_start(s_sb[:, 2 * BC:4 * BC], sP[:, 2 * BC:4 * BC])

            bias_sb = wpool.tile([P, MT], f32, tag="bias", name="bias_sb")
            nc.gpsimd.dma_start(bias_sb[:], bias.rearrange("(m p) -> p m", p=P))

            # delta weights 0.1*Wb, cast on the (idle-early) scalar engine
            wd = wpool.tile([P, KT_S * N], f16, tag="wd", name="wd")
            for j in range(KT_S):
                nc.scalar.mul(wd[:, j * N:(j + 1) * N],
                              w_sb[:, (KT_X + j) * N:(KT_X + j + 1) * N], DT)

            # ---- persistent PSUM accumulators ------------------------------
            ps = [psump.tile([P, BC], f32, tag=f"ps{m}", name=f"ps{m}")
                  for m in range(MT)]

            # HAM warm-up (and bridges the input-DMA latency)
            for r in range(10):
                nc.tensor.matmul(
                    ps[r % MT][:, 0:CHUNK],
                    lhsT=junk[:, 0:P], rhs=junk[:, 0:CHUNK],
                    start=True, stop=True, skip_group_check=True,
                )

            def wslice(t, j, m):
                return t[:, j * N + m * P: j * N + (m + 1) * P]

            # init: psum = x @ Wt + s0 @ Wb  (fp16). x-part j-outer (follows
            # DMA arrival); s-part m-outer so ps[m] releases early for round 0.
            for j in range(KT_X):
                for m in range(MT):
                    for c in range(NCH):
                        nc.tensor.matmul(
                            ps[m][:, c * CHUNK:(c + 1) * CHUNK],
                            lhsT=wslice(w_sb, j, m),
                            rhs=x_sb[:, j * BC + c * CHUNK:
                                     j * BC + (c + 1) * CHUNK],
                            start=(j == 0), stop=False, skip_group_check=True,
                        )
            for m in range(MT):
                for j in range(KT_S):
                    for c in range(NCH):
                        nc.tensor.matmul(
                            ps[m][:, c * CHUNK:(c + 1) * CHUNK],
                            lhsT=wslice(w_sb, KT_X + j, m),
                            rhs=s_sb[:, j * BC + c * CHUNK:
                                     j * BC + (c + 1) * CHUNK],
                            start=False, stop=False, skip_group_check=True,
                        )

            # ---- unfolds ---------------------------------------------------
            for k in range(UNFOLDS):
                last = k == UNFOLDS - 1
                f_t = [fpool.tile([P, BC], f16, tag=f"f{m}", name=f"f{k}_{m}")
                       for m in range(MT)]
                u_t = [hpool.tile([P, BC], f16, tag=f"u{m}", name=f"u{k}_{m}")
                       for m in range(MT)]
                tmp_t = [tmpp.tile([P, BC], f16, tag=f"tmp{j}",
                                   name=f"tmp{k}_{j}")
                         for j in range(MT)]
                for m in range(MT):
                    nc.scalar.activation(
                        f_t[m][:], ps[m][:], TANH,
                        bias=bias_sb[:, m:m + 1], scale=1.0,
                    )
                    # tmp = f - s (TT, 684ns)
                    nc.vector.tensor_tensor(
                        tmp_t[m][:], f_t[m][:],
                        s_sb[:, m * BC:(m + 1) * BC], SUB)
                    # u = 0.1*tmp ; s += u via DMA-accumulate
                    nc.vector.tensor_scalar_mul(u_t[m][:], tmp_t[m][:], DT)
                if not last:
                    for m in range(MT):
                        nc.gpsimd.dma_start(s_sb[:, m * BC:(m + 1) * BC],
                                            u_t[m][:], accum_op=ADD)
                    # psum += tmp @ (0.1*Wb), j-outer (ps[m] releases at 76%)
                    for j in range(KT_S):
                        for m in range(MT):
                            for c in range(NCH):
                                nc.tensor.matmul(
                                    ps[m][:, c * CHUNK:(c + 1) * CHUNK],
                                    lhsT=wslice(wd, j, m),
                                    rhs=tmp_t[j][:, c * CHUNK:(c + 1) * CHUNK],
                                    start=False,
                                    stop=(k == UNFOLDS - 2 and j == KT_S - 1),
                                    skip_group_check=True,
                                )
                else:
                    o_t = [opool.tile([P, BC], f16, tag=f"o{m}",
                                      name=f"o{m}") for m in range(MT)]
                    for m in range(MT):
                        # out = s + u  (don't wait for the DMA-accumulate)
                        nc.vector.tensor_tensor(
                            o_t[m][:], s_sb[:, m * BC:(m + 1) * BC],
                            u_t[m][:], ADD)
                        out_eng = (nc.sync, nc.scalar, nc.sync, nc.scalar)[m]
                        out_eng.dma_start(outT[m * P:(m + 1) * P, :], o_t[m][:])

    nc.compile()
    return nc


def _get_nc():
    global _compiled_nc
    if _compiled_nc is None:
        _compiled_nc = _build_nc()
    return _compiled_nc


def make_in_maps(x, s, W, b):
    """Shard + pack host-side: x/s transposed to (128, k*1024) fp16 with
    k-tiles side by side; W packed to (128, 8*512) fp16 k-tile-major."""
    xT = np.ascontiguousarray(x.T)   # (D, B)
    sT = np.ascontiguousarray(s.T)   # (N, B)
    Wp = np.ascontiguousarray(
        W.reshape(8, P, N).transpose(1, 0, 2).reshape(P, -1)
    ).astype(np.float16)
    in_maps = []
    for c in range(NCORES):
        sl = slice(c * BC, (c + 1) * BC)
        xs = xT[:, sl].reshape(KT_X, P, BC).transpose(1, 0, 2).reshape(P, -1)
        ss = sT[:, sl].reshape(KT_S, P, BC).transpose(1, 0, 2).reshape(P, -1)
        in_maps.append({
            "xP": np.ascontiguousarray(xs).astype(np.float16),
            "sP": np.ascontiguousarray(ss).astype(np.float16),
            "Wp": Wp,
            "bias": b,
        })
    return in_maps


def kernel(**inputs):
    from concourse.bass_utils import run_bass_kernel_spmd

    x = np.asarray(inputs["inputs"], dtype=np.float32)
    s = np.asarray(inputs["state"], dtype=np.float32)
    W = np.ascontiguousarray(np.asarray(inputs["W"], dtype=np.float32))
    b = np.ascontiguousarray(np.asarray(inputs["bias"], dtype=np.float32))

    in_maps = make_in_maps(x, s, W, b)
    nc = _get_nc()
    res = run_bass_kernel_spmd(nc, in_maps, list(range(NCORES))).results
    outT = np.concatenate([res[c]["outT"] for c in range(NCORES)], axis=1)
    out = np.ascontiguousarray(outT.T).astype(np.float32)
    return (out, out)


# revision 12
# speedup vs baseline: 1.3157x; 1.0184x over previous
"""CTRNN cell (6 Euler unfolds) on 8 Trainium2 NeuronCores.

Math (per unfold, 6x):
    f     = tanh([x, s] @ W + b)
    s_new = s + 0.1 * (f - s)

v3 strategy (measured-rate driven):
  - Data-parallel over batch: B=8192 -> 1024 rows/core, no cross-core comm.
  - All tensors fp16. Measured: fp16 MMs hit the 216ns/512-col streaming
    rate with FWL weight loads fully hidden; fp8 DoubleRow measured 1.0x
    (dropped). DVE rates: tensor_tensor 684ns, tensor_scalar 427ns,
    scalar_tensor_tensor 1218ns (avoided) per (128,1024) tile.
  - Delta form: psum holds x@Wt + s_k@Wb; per unfold
    psum += tmp_k @ (0.1*Wb), tmp_k = f_k - s_k.  7-matmul FLOP floor.
  - Per round per m-tile:
      tmp = f - s        DVE tensor_tensor subtract (684ns, short path)
      u   = 0.1*tmp      DVE tensor_scalar_mul (427ns)
      s  += u            gpsimd DMA-accumulate (rides idle DMA engines)
    DVE/round 4.4us, ACT 4.45us, both under the 6.9us PE round.
  - Delta rounds j-outer so ps[0] releases at 76% of the round; next
    round's first MMs need only tmp[0] whose path is ACT+TT = 1.8us.
  - Output = final s update, shipped fp16, upcast to f32 on host
    (tolerance 2e-2; fp16 quantization ~5e-4).
"""

import numpy as np

UNFOLDS = 6
DT = 0.1
B, D, N = 8192, 512, 512
NCORES = 8
BC = B // NCORES          # batch rows per core
CHUNK = 512               # matmul moving free dim (one PSUM bank of f32)
NCH = BC // CHUNK         # 2
P = 128
KT_X = D // P             # 4 k-tiles of W_top
KT_S = N // P             # 4 k-tiles of W_bot
MT = N // P               # 4 m-tiles of the output dim

_compiled_nc = None


def _build_nc():
    import concourse.bass as bass  # noqa: F401
    import concourse.bacc as bacc
    import concourse.tile as tile
    from concourse import mybir

    f32 = mybir.dt.float32
    f16 = mybir.dt.float16
    ADD = mybir.AluOpType.add
    SUB = mybir.AluOpType.subtract
    TANH = mybir.ActivationFunctionType.Tanh

    nc = bacc.Bacc("TRN2", target_bir_lowering=False, debug=False)

    xP = nc.dram_tensor("xP", [P, D * BC // P], f16, kind="ExternalInput").ap()
    sP = nc.dram_tensor("sP", [P, N * BC // P], f16, kind="ExternalInput").ap()
    Wp_d = nc.dram_tensor("Wp", [P, (D + N) * N // P], f16,
                          kind="ExternalInput").ap()
    bias = nc.dram_tensor("bias", [N], f32, kind="ExternalInput").ap()
    outT = nc.dram_tensor("outT", [N, BC], f16, kind="ExternalOutput").ap()

    with tile.TileContext(nc) as tc:
        with (
            tc.tile_pool(name="weights", bufs=1) as wpool,
            tc.tile_pool(name="data", bufs=1) as data,
            tc.tile_pool(name="tmp", bufs=2) as tmpp,
            tc.tile_pool(name="fpool", bufs=2) as fpool,
            tc.tile_pool(name="hpool", bufs=2) as hpool,
            tc.tile_pool(name="opool", bufs=1) as opool,
            tc.tile_pool(name="psum", bufs=1, space="PSUM") as psump,
        ):
            # ---- input DMAs ------------------------------------------------
            junk = wpool.tile([P, CHUNK], f16, tag="junk", name="junk")
            nc.gpsimd.memset(junk[:], 0)

            # W ships Wb-first (s-part matmuls + wd casts unblock early);
            # fine-grained leading chunks so the first matmuls start ASAP.
            w_sb = data.tile([P, (D + N) * N // P], f16, tag="w", name="w_sb")
            nc.scalar.dma_start(w_sb[:, 4 * N:5 * N], Wp_d[:, 4 * N:5 * N])
            nc.scalar.dma_start(w_sb[:, 5 * N:8 * N], Wp_d[:, 5 * N:8 * N])
            nc.scalar.dma_start(w_sb[:, 0:4 * N], Wp_d[:, 0:4 * N])

            s_sb = data.tile([P, N * BC // P], f16, tag="s", name="s_sb")
            nc.sync.dma_start(s_sb[:, 0:BC], sP[:, 0:BC])
            nc.sync.dma_start(s_sb[:, BC:2 * BC], sP[:, BC:2 * BC])
            nc.sync.dma_start(s_sb[:, 2 * BC:4 * BC], sP[:, 2 * BC:4 * BC])

            x_sb = data.tile([P, D * BC // P], f16, tag="x", name="x_sb")
            nc.gpsimd.dma_start(x_sb[:, 0:2 * BC], xP[:, 0:2 * BC])
            nc.gpsimd.dma_start(x_sb[:, 2 * BC:4 * BC], xP[:, 2 * BC:4 * BC])

            bias_sb = wpool.tile([P, MT], f32, tag="bias", name="bias_sb")
            nc.gpsimd.dma_start(bias_sb[:], bias.rearrange("(m p) -> p m", p=P))

            # delta weights 0.1*Wb, cast on the (idle-early) scalar engine
            wd = wpool.tile([P, KT_S * N], f16, tag="wd", name="wd")
            for j in range(KT_S):
                nc.scalar.mul(wd[:, j * N:(j + 1) * N],
                              w_sb[:, (KT_X + j) * N:(KT_X + j + 1) * N], DT)

            # ---- persistent PSUM accumulators ------------------------------
            ps = [psump.tile([P, BC], f32, tag=f"ps{m}", name=f"ps{m}")
                  for m in range(MT)]

            # HAM warm-up (and bridges the input-DMA latency)
            for r in range(10):
                nc.tensor.matmul(
                    ps[r % MT][:, 0:CHUNK],
                    lhsT=junk[:, 0:P], rhs=junk[:, 0:CHUNK],
                    start=True, stop=True, skip_group_check=True,
                )

            def wslice(t, j, m):
                return t[:, j * N + m * P: j * N + (m + 1) * P]

            # init: psum = s0 @ Wb + x @ Wt (fp16). s-part first, j-outer
            # (follows DMA arrival); x-part m-outer so ps[m] releases early
            # for round 0's tanh.
            for j in range(KT_S):
                for m in range(MT):
                    for c in range(NCH):
                        nc.tensor.matmul(
                            ps[m][:, c * CHUNK:(c + 1) * CHUNK],
                            lhsT=wslice(w_sb, KT_X + j, m),
                            rhs=s_sb[:, j * BC + c * CHUNK:
                                     j * BC + (c + 1) * CHUNK],
                            start=(j == 0), stop=False, skip_group_check=True,
                        )
            for m in range(MT):
                for j in range(KT_X):
                    for c in range(NCH):
                        nc.tensor.matmul(
                            ps[m][:, c * CHUNK:(c + 1) * CHUNK],
                            lhsT=wslice(w_sb, j, m),
                            rhs=x_sb[:, j * BC + c * CHUNK:
                                     j * BC + (c + 1) * CHUNK],
                            start=False, stop=False, skip_group_check=True,
                        )

            # ---- unfolds ---------------------------------------------------
            for k in range(UNFOLDS):
                last = k == UNFOLDS - 1
                f_t = [fpool.tile([P, BC], f16, tag=f"f{m}", name=f"f{k}_{m}")
                       for m in range(MT)]
                u_t = [hpool.tile([P, BC], f16, tag=f"u{m}", name=f"u{k}_{m}")
                       for m in range(MT)]
                tmp_t = [tmpp.tile([P, BC], f16, tag=f"tmp{j}",
                                   name=f"tmp{k}_{j}")
                         for j in range(MT)]
                for m in range(MT):
                    nc.scalar.activation(
                        f_t[m][:], ps[m][:], TANH,
                        bias=bias_sb[:, m:m + 1], scale=1.0,
                    )
                    # tmp = f - s (TT, 684ns)
                    nc.vector.tensor_tensor(
                        tmp_t[m][:], f_t[m][:],
                        s_sb[:, m * BC:(m + 1) * BC], SUB)
                    # u = 0.1*tmp ; s += u via DMA-accumulate
                    if not last or m == MT - 1:
                        nc.vector.tensor_scalar_mul(u_t[m][:], tmp_t[m][:], DT)
                if not last:
                    for m in range(MT):
                        nc.gpsimd.dma_start(s_sb[:, m * BC:(m + 1) * BC],
                                            u_t[m][:], accum_op=ADD)
                    # psum += tmp @ (0.1*Wb).  Hybrid order: j0/j1 across all
                    # m, then (j2,j3) per m — ps[0] completes at 56% of the
                    # round so the next round's tanh+tmp path fully hides.
                    order = ([(j, m) for j in (0, 1) for m in range(MT)] +
                             [(j, m) for m in range(MT) for j in (2, 3)])
                    for j, m in order:
                        for c in range(NCH):
                            nc.tensor.matmul(
                                ps[m][:, c * CHUNK:(c + 1) * CHUNK],
                                lhsT=wslice(wd, j, m),
                                rhs=tmp_t[j][:, c * CHUNK:(c + 1) * CHUNK],
                                start=False,
                                stop=(k == UNFOLDS - 2 and j == 3),
                                skip_group_check=True,
                            )
                else:
                    o_t = [opool.tile([P, BC], f16, tag=f"o{m}",
                                      name=f"o{m}") for m in range(MT)]
                    for m in range(MT):
                        # out = s + u.  Tiles 0-2: u is already in o via
                        # ts_mul retarget + gpsimd DMA-accumulate of s
                        # (latency hides behind later tiles).  Tile 3 (the
                        # last chain) stays on the lower-latency DVE path.
                        if m < MT - 1:
                            nc.vector.tensor_scalar_mul(
                                o_t[m][:], tmp_t[m][:], DT)
                            nc.gpsimd.dma_start(
                                o_t[m][:], s_sb[:, m * BC:(m + 1) * BC],
                                accum_op=ADD)
                        else:
                            nc.vector.tensor_tensor(
                                o_t[m][:], s_sb[:, m * BC:(m + 1) * BC],
                                u_t[m][:], ADD)
                        out_eng = (nc.sync, nc.scalar, nc.sync, nc.scalar)[m]
                        out_eng.dma_start(outT[m * P:(m + 1) * P, :], o_t[m][:])

    nc.compile()
    return nc


def _get_nc():
    global _compiled_nc
    if _compiled_nc is None:
        _compiled_nc = _build_nc()
    return _compiled_nc


def make_in_maps(x, s, W, b):
    """Shard + pack host-side: x/s transposed to (128, k*1024) fp16 with
    k-tiles side by side; W packed to (128, 8*512) fp16 k-tile-major."""
    xT = np.ascontiguousarray(x.T)   # (D, B)
    sT = np.ascontiguousarray(s.T)   # (N, B)
    Wp = np.ascontiguousarray(
        W.reshape(8, P, N).transpose(1, 0, 2).reshape(P, -1)
    ).astype(np.float16)
    in_maps = []
    for c in range(NCORES):
        sl = slice(c * BC, (c + 1) * BC)
        xs = xT[:, sl].reshape(KT_X, P, BC).transpose(1, 0, 2).reshape(P, -1)
        ss = sT[:, sl].reshape(KT_S, P, BC).transpose(1, 0, 2).reshape(P, -1)
        in_maps.append({
            "xP": np.ascontiguousarray(xs).astype(np.float16),
            "sP": np.ascontiguousarray(ss).astype(np.float16),
            "Wp": Wp,
            "bias": b,
        })
    return in_maps


def kernel(**inputs):
    from concourse.bass_utils import run_bass_kernel_spmd

    x = np.asarray(inputs["inputs"], dtype=np.float32)
    s = np.asarray(inputs["state"], dtype=np.float32)
    W = np.ascontiguousarray(np.asarray(inputs["W"], dtype=np.float32))
    b = np.ascontiguousarray(np.asarray(inputs["bias"], dtype=np.float32))

    in_maps = make_in_maps(x, s, W, b)
    nc = _get_nc()
    res = run_bass_kernel_spmd(nc, in_maps, list(range(NCORES))).results
    outT = np.concatenate([res[c]["outT"] for c in range(NCORES)], axis=1)
    out = np.ascontiguousarray(outT.T).astype(np.float32)
    return (out, out)


# revision 15
# speedup vs baseline: 1.3624x; 1.0355x over previous
"""CTRNN cell (6 Euler unfolds) on 8 Trainium2 NeuronCores.

Math (per unfold, 6x):
    f     = tanh([x, s] @ W + b)
    s_new = s + 0.1 * (f - s)

v3 strategy (measured-rate driven):
  - Data-parallel over batch: B=8192 -> 1024 rows/core, no cross-core comm.
  - All tensors fp16. Measured: fp16 MMs hit the 216ns/512-col streaming
    rate with FWL weight loads fully hidden; fp8 DoubleRow measured 1.0x
    (dropped). DVE rates: tensor_tensor 684ns, tensor_scalar 427ns,
    scalar_tensor_tensor 1218ns (avoided) per (128,1024) tile.
  - Delta form: psum holds x@Wt + s_k@Wb; per unfold
    psum += tmp_k @ (0.1*Wb), tmp_k = f_k - s_k.  7-matmul FLOP floor.
  - Per round per m-tile:
      tmp = f - s        DVE tensor_tensor subtract (684ns, short path)
      u   = 0.1*tmp      DVE tensor_scalar_mul (427ns)
      s  += u            gpsimd DMA-accumulate (rides idle DMA engines)
    DVE/round 4.4us, ACT 4.45us, both under the 6.9us PE round.
  - Delta rounds j-outer so ps[0] releases at 76% of the round; next
    round's first MMs need only tmp[0] whose path is ACT+TT = 1.8us.
  - Output = final s update, shipped fp16, upcast to f32 on host
    (tolerance 2e-2; fp16 quantization ~5e-4).
"""

import numpy as np

UNFOLDS = 6
DT = 0.1
B, D, N = 8192, 512, 512
NCORES = 8
BC = B // NCORES          # batch rows per core
CHUNK = 512               # matmul moving free dim (one PSUM bank of f32)
NCH = BC // CHUNK         # 2
P = 128
KT_X = D // P             # 4 k-tiles of W_top
KT_S = N // P             # 4 k-tiles of W_bot
MT = N // P               # 4 m-tiles of the output dim

_compiled_nc = None


def _build_nc():
    import concourse.bass as bass  # noqa: F401
    import concourse.bacc as bacc
    import concourse.tile as tile
    from concourse import mybir

    f32 = mybir.dt.float32
    f16 = mybir.dt.float16
    ADD = mybir.AluOpType.add
    SUB = mybir.AluOpType.subtract
    TANH = mybir.ActivationFunctionType.Tanh

    nc = bacc.Bacc("TRN2", target_bir_lowering=False, debug=False)

    xP = nc.dram_tensor("xP", [P, D * BC // P], f16, kind="ExternalInput").ap()
    sP = nc.dram_tensor("sP", [P, N * BC // P], f16, kind="ExternalInput").ap()
    Wp_d = nc.dram_tensor("Wp", [P, (D + N) * N // P], f16,
                          kind="ExternalInput").ap()
    bias = nc.dram_tensor("bias", [N], f32, kind="ExternalInput").ap()
    outT = nc.dram_tensor("outT", [N, BC], f16, kind="ExternalOutput").ap()

    with tile.TileContext(nc) as tc:
        with (
            tc.tile_pool(name="weights", bufs=1) as wpool,
            tc.tile_pool(name="data", bufs=1) as data,
            tc.tile_pool(name="tmp", bufs=2) as tmpp,
            tc.tile_pool(name="fpool", bufs=2) as fpool,
            tc.tile_pool(name="hpool", bufs=2) as hpool,
            tc.tile_pool(name="opool", bufs=1) as opool,
            tc.tile_pool(name="psum", bufs=1, space="PSUM") as psump,
        ):
            # ---- input DMAs ------------------------------------------------
            junk = wpool.tile([P, CHUNK], f16, tag="junk", name="junk")
            nc.gpsimd.memset(junk[:], 0)

            # Input load is DMA-descriptor-rate bound (~130GB/s/queue at 4KB
            # descriptors).  Use 4 queues, 4KB/partition descriptors: Wb on
            # scalar, Wt on vector, x on gpsimd, s on sync (small leading
            # chunk so the s-part matmuls start ASAP).
            w_sb = data.tile([P, (D + N) * N // P], f16, tag="w", name="w_sb")
            nc.scalar.dma_start(w_sb[:, 4 * N:8 * N], Wp_d[:, 4 * N:8 * N])
            nc.scalar.dma_start(w_sb[:, 0:4 * N], Wp_d[:, 0:4 * N])

            s_sb = data.tile([P, N * BC // P], f16, tag="s", name="s_sb")
            nc.sync.dma_start(s_sb[:, 0:BC], sP[:, 0:BC])
            nc.sync.dma_start(s_sb[:, BC:2 * BC], sP[:, BC:2 * BC])
            nc.sync.dma_start(s_sb[:, 2 * BC:4 * BC], sP[:, 2 * BC:4 * BC])

            x_sb = data.tile([P, D * BC // P], f16, tag="x", name="x_sb")
            nc.gpsimd.dma_start(x_sb[:, 0:2 * BC], xP[:, 0:2 * BC])
            nc.gpsimd.dma_start(x_sb[:, 2 * BC:4 * BC], xP[:, 2 * BC:4 * BC])

            bias_sb = wpool.tile([P, MT], f32, tag="bias", name="bias_sb")
            nc.gpsimd.dma_start(bias_sb[:], bias.rearrange("(m p) -> p m", p=P))

            # delta weights 0.1*Wb, cast on the (idle-early) scalar engine
            wd = wpool.tile([P, KT_S * N], f16, tag="wd", name="wd")
            for j in range(KT_S):
                nc.scalar.mul(wd[:, j * N:(j + 1) * N],
                              w_sb[:, (KT_X + j) * N:(KT_X + j + 1) * N], DT)

            # ---- persistent PSUM accumulators ------------------------------
            ps = [psump.tile([P, BC], f32, tag=f"ps{m}", name=f"ps{m}")
                  for m in range(MT)]

            # HAM warm-up (and bridges the input-DMA latency)
            for r in range(8):
                nc.tensor.matmul(
                    ps[r % MT][:, 0:CHUNK],
                    lhsT=junk[:, 0:P], rhs=junk[:, 0:CHUNK],
                    start=True, stop=True, skip_group_check=True,
                )

            def wslice(t, j, m):
                return t[:, j * N + m * P: j * N + (m + 1) * P]

            # init: psum = s0 @ Wb + x @ Wt (fp16). s-part first, j-outer
            # (follows DMA arrival); x-part m-outer so ps[m] releases early
            # for round 0's tanh.
            for j in range(KT_S):
                for m in range(MT):
                    for c in range(NCH):
                        nc.tensor.matmul(
                            ps[m][:, c * CHUNK:(c + 1) * CHUNK],
                            lhsT=wslice(w_sb, KT_X + j, m),
                            rhs=s_sb[:, j * BC + c * CHUNK:
                                     j * BC + (c + 1) * CHUNK],
                            start=(j == 0), stop=False, skip_group_check=True,
                        )
            for m in range(MT):
                for j in range(KT_X):
                    for c in range(NCH):
                        nc.tensor.matmul(
                            ps[m][:, c * CHUNK:(c + 1) * CHUNK],
                            lhsT=wslice(w_sb, j, m),
                            rhs=x_sb[:, j * BC + c * CHUNK:
                                     j * BC + (c + 1) * CHUNK],
                            start=False, stop=False, skip_group_check=True,
                        )

            # ---- unfolds ---------------------------------------------------
            for k in range(UNFOLDS):
                last = k == UNFOLDS - 1
                f_t = [fpool.tile([P, BC], f16, tag=f"f{m}", name=f"f{k}_{m}")
                       for m in range(MT)]
                u_t = [hpool.tile([P, BC], f16, tag=f"u{m}", name=f"u{k}_{m}")
                       for m in range(MT)]
                tmp_t = [tmpp.tile([P, BC], f16, tag=f"tmp{j}",
                                   name=f"tmp{k}_{j}")
                         for j in range(MT)]
                for m in range(MT):
                    nc.scalar.activation(
                        f_t[m][:], ps[m][:], TANH,
                        bias=bias_sb[:, m:m + 1], scale=1.0,
                    )
                    # tmp = f - s (TT, 684ns)
                    nc.vector.tensor_tensor(
                        tmp_t[m][:], f_t[m][:],
                        s_sb[:, m * BC:(m + 1) * BC], SUB)
                    # u = 0.1*tmp ; s += u via DMA-accumulate
                    if not last or m == MT - 1:
                        nc.vector.tensor_scalar_mul(u_t[m][:], tmp_t[m][:], DT)
                if not last:
                    for m in range(MT):
                        nc.gpsimd.dma_start(s_sb[:, m * BC:(m + 1) * BC],
                                            u_t[m][:], accum_op=ADD)
                    # psum += tmp @ (0.1*Wb).  Hybrid order: j0/j1 across all
                    # m, then (j2,j3) per m — ps[0] completes at 56% of the
                    # round so the next round's tanh+tmp path fully hides.
                    order = ([(j, m) for j in (0, 1) for m in range(MT)] +
                             [(j, m) for m in range(MT) for j in (2, 3)])
                    for j, m in order:
                        for c in range(NCH):
                            nc.tensor.matmul(
                                ps[m][:, c * CHUNK:(c + 1) * CHUNK],
                                lhsT=wslice(wd, j, m),
                                rhs=tmp_t[j][:, c * CHUNK:(c + 1) * CHUNK],
                                start=False,
                                stop=(k == UNFOLDS - 2 and j == 3),
                                skip_group_check=True,
                            )
                else:
                    o_t = [opool.tile([P, BC], f16, tag=f"o{m}",
                                      name=f"o{m}") for m in range(MT)]
                    for m in range(MT):
                        # out = s + u.  Tiles 0-2: u is already in o via
                        # ts_mul retarget + gpsimd DMA-accumulate of s
                        # (latency hides behind later tiles).  Tile 3 (the
                        # last chain) stays on the lower-latency DVE path.
                        if m < MT - 1:
                            nc.vector.tensor_scalar_mul(
                                o_t[m][:], tmp_t[m][:], DT)
                            nc.gpsimd.dma_start(
                                o_t[m][:], s_sb[:, m * BC:(m + 1) * BC],
                                accum_op=ADD)
                        else:
                            nc.vector.tensor_tensor(
                                o_t[m][:], s_sb[:, m * BC:(m + 1) * BC],
                                u_t[m][:], ADD)
                        out_eng = (nc.sync, nc.scalar, nc.sync, nc.scalar)[m]
                        out_eng.dma_start(outT[m * P:(m + 1) * P, :], o_t[m][:])

    nc.compile()
    return nc


def _get_nc():
    global _compiled_nc
    if _compiled_nc is None:
        _compiled_nc = _build_nc()
    return _compiled_nc


def make_in_maps(x, s, W, b):
    """Shard + pack host-side: x/s transposed to (128, k*1024) fp16 with
    k-tiles side by side; W packed to (128, 8*512) fp16 k-tile-major."""
    xT = np.ascontiguousarray(x.T)   # (D, B)
    sT = np.ascontiguousarray(s.T)   # (N, B)
    Wp = np.ascontiguousarray(
        W.reshape(8, P, N).transpose(1, 0, 2).reshape(P, -1)
    ).astype(np.float16)
    in_maps = []
    for c in range(NCORES):
        sl = slice(c * BC, (c + 1) * BC)
        xs = xT[:, sl].reshape(KT_X, P, BC).transpose(1, 0, 2).reshape(P, -1)
        ss = sT[:, sl].reshape(KT_S, P, BC).transpose(1, 0, 2).reshape(P, -1)
        in_maps.append({
            "xP": np.ascontiguousarray(xs).astype(np.float16),
            "sP": np.ascontiguousarray(ss).astype(np.float16),
            "Wp": Wp,
            "bias": b,
        })
    return in_maps


def kernel(**inputs):
    from concourse.bass_utils import run_bass_kernel_spmd

    x = np.asarray(inputs["inputs"], dtype=np.float32)
    s = np.asarray(inputs["state"], dtype=np.float32)
    W = np.ascontiguousarray(np.asarray(inputs["W"], dtype=np.float32))
    b = np.ascontiguousarray(np.asarray(inputs["bias"], dtype=np.float32))

    in_maps = make_in_maps(x, s, W, b)
    nc = _get_nc()
    res = run_bass_kernel_spmd(nc, in_maps, list(range(NCORES))).results
    outT = np.concatenate([res[c]["outT"] for c in range(NCORES)], axis=1)
    out = np.ascontiguousarray(outT.T).astype(np.float32)
    return (out, out)


# revision 19
# speedup vs baseline: 1.4103x; 1.0351x over previous
"""CTRNN cell (6 Euler unfolds) on 8 Trainium2 NeuronCores.

Math (per unfold, 6x):
    f     = tanh([x, s] @ W + b)
    s_new = s + 0.1 * (f - s)

v3 strategy (measured-rate driven):
  - Data-parallel over batch: B=8192 -> 1024 rows/core, no cross-core comm.
  - All tensors fp16. Measured: fp16 MMs hit the 216ns/512-col streaming
    rate with FWL weight loads fully hidden; fp8 DoubleRow measured 1.0x
    (dropped). DVE rates: tensor_tensor 684ns, tensor_scalar 427ns,
    scalar_tensor_tensor 1218ns (avoided) per (128,1024) tile.
  - Delta form: psum holds x@Wt + s_k@Wb; per unfold
    psum += tmp_k @ (0.1*Wb), tmp_k = f_k - s_k.  7-matmul FLOP floor.
  - Per round per m-tile:
      tmp = f - s        DVE tensor_tensor subtract (684ns, short path)
      u   = 0.1*tmp      DVE tensor_scalar_mul (427ns)
      s  += u            gpsimd DMA-accumulate (rides idle DMA engines)
    DVE/round 4.4us, ACT 4.45us, both under the 6.9us PE round.
  - Delta rounds j-outer so ps[0] releases at 76% of the round; next
    round's first MMs need only tmp[0] whose path is ACT+TT = 1.8us.
  - Output = final s update, shipped fp16, upcast to f32 on host
    (tolerance 2e-2; fp16 quantization ~5e-4).
"""

import numpy as np

UNFOLDS = 6
DT = 0.1
B, D, N = 8192, 512, 512
NCORES = 8
BC = B // NCORES          # batch rows per core
CHUNK = 512               # matmul moving free dim (one PSUM bank of f32)
NCH = BC // CHUNK         # 2
P = 128
KT_X = D // P             # 4 k-tiles of W_top
KT_S = N // P             # 4 k-tiles of W_bot
MT = N // P               # 4 m-tiles of the output dim

_compiled_nc = None


def _build_nc():
    import concourse.bass as bass  # noqa: F401
    import concourse.bacc as bacc
    import concourse.tile as tile
    from concourse import mybir

    f32 = mybir.dt.float32
    f16 = mybir.dt.float16
    ADD = mybir.AluOpType.add
    SUB = mybir.AluOpType.subtract
    TANH = mybir.ActivationFunctionType.Tanh

    nc = bacc.Bacc("TRN2", target_bir_lowering=False, debug=False)

    xP = nc.dram_tensor("xP", [P, D * BC // P], f16, kind="ExternalInput").ap()
    sP = nc.dram_tensor("sP", [P, N * BC // P], f16, kind="ExternalInput").ap()
    Wp_d = nc.dram_tensor("Wp", [P, (D + N) * N // P], f16,
                          kind="ExternalInput").ap()
    bias = nc.dram_tensor("bias", [N], f32, kind="ExternalInput").ap()
    outT = nc.dram_tensor("outT", [N, BC], f16, kind="ExternalOutput").ap()

    with tile.TileContext(nc) as tc:
        with (
            tc.tile_pool(name="weights", bufs=1) as wpool,
            tc.tile_pool(name="data", bufs=1) as data,
            tc.tile_pool(name="tmp", bufs=2) as tmpp,
            tc.tile_pool(name="fpool", bufs=2) as fpool,
            tc.tile_pool(name="hpool", bufs=2) as hpool,
            tc.tile_pool(name="opool", bufs=1) as opool,
            tc.tile_pool(name="psum", bufs=1, space="PSUM") as psump,
        ):
            # ---- input DMAs ------------------------------------------------
            junk = wpool.tile([P, CHUNK], f16, tag="junk", name="junk")
            nc.gpsimd.memset(junk[:], 0)

            # Input load is DMA-descriptor-rate bound (~130GB/s/queue at 4KB
            # descriptors).  Use 4 queues, 4KB/partition descriptors: Wb on
            # scalar, Wt on vector, x on gpsimd, s on sync (small leading
            # chunk so the s-part matmuls start ASAP).
            w_sb = data.tile([P, (D + N) * N // P], f16, tag="w", name="w_sb")
            nc.scalar.dma_start(w_sb[:, 4 * N:8 * N], Wp_d[:, 4 * N:8 * N])
            nc.scalar.dma_start(w_sb[:, 0:4 * N], Wp_d[:, 0:4 * N])

            s_sb = data.tile([P, N * BC // P], f16, tag="s", name="s_sb")
            nc.sync.dma_start(s_sb[:, 0:2 * BC], sP[:, 0:2 * BC])
            nc.sync.dma_start(s_sb[:, 2 * BC:4 * BC], sP[:, 2 * BC:4 * BC])

            x_sb = data.tile([P, D * BC // P], f16, tag="x", name="x_sb")
            nc.gpsimd.dma_start(x_sb[:, 0:2 * BC], xP[:, 0:2 * BC])
            nc.gpsimd.dma_start(x_sb[:, 2 * BC:4 * BC], xP[:, 2 * BC:4 * BC])

            bias_sb = wpool.tile([P, MT], f32, tag="bias", name="bias_sb")
            nc.gpsimd.dma_start(bias_sb[:], bias.rearrange("(m p) -> p m", p=P))

            # delta weights 0.1*Wb, cast on the (idle-early) scalar engine
            wd = wpool.tile([P, KT_S * N], f16, tag="wd", name="wd")
            for j in range(KT_S):
                nc.scalar.mul(wd[:, j * N:(j + 1) * N],
                              w_sb[:, (KT_X + j) * N:(KT_X + j + 1) * N], DT)

            # ---- persistent PSUM accumulators ------------------------------
            ps = [psump.tile([P, BC], f32, tag=f"ps{m}", name=f"ps{m}")
                  for m in range(MT)]

            # HAM warm-up (and bridges the input-DMA latency)
            for r in range(8):
                nc.tensor.matmul(
                    ps[r % MT][:, 0:CHUNK],
                    lhsT=junk[:, 0:P], rhs=junk[:, 0:CHUNK],
                    start=True, stop=True, skip_group_check=True,
                )

            def wslice(t, j, m):
                return t[:, j * N + m * P: j * N + (m + 1) * P]

            # init: psum = s0 @ Wb + x @ Wt (fp16). s-part first, j-outer
            # (follows DMA arrival); x-part m-outer so ps[m] releases early
            # for round 0's tanh.
            for j in range(KT_S):
                for m in range(MT):
                    for c in range(NCH):
                        nc.tensor.matmul(
                            ps[m][:, c * CHUNK:(c + 1) * CHUNK],
                            lhsT=wslice(w_sb, KT_X + j, m),
                            rhs=s_sb[:, j * BC + c * CHUNK:
                                     j * BC + (c + 1) * CHUNK],
                            start=(j == 0), stop=False, skip_group_check=True,
                        )
            for m in range(MT):
                for j in range(KT_X):
                    for c in range(NCH):
                        nc.tensor.matmul(
                            ps[m][:, c * CHUNK:(c + 1) * CHUNK],
                            lhsT=wslice(w_sb, j, m),
                            rhs=x_sb[:, j * BC + c * CHUNK:
                                     j * BC + (c + 1) * CHUNK],
                            start=False, stop=False, skip_group_check=True,
                        )

            # ---- unfolds ---------------------------------------------------
            for k in range(UNFOLDS):
                last = k == UNFOLDS - 1
                f_t = [fpool.tile([P, BC], f16, tag=f"f{m}", name=f"f{k}_{m}")
                       for m in range(MT)]
                u_t = [hpool.tile([P, BC], f16, tag=f"u{m}", name=f"u{k}_{m}")
                       for m in range(MT)]
                tmp_t = [tmpp.tile([P, BC], f16, tag=f"tmp{j}",
                                   name=f"tmp{k}_{j}")
                         for j in range(MT)]
                for m in range(MT):
                    nc.scalar.activation(
                        f_t[m][:], ps[m][:], TANH,
                        bias=bias_sb[:, m:m + 1], scale=1.0,
                    )
                    # tmp = f - s (TT, 684ns)
                    nc.vector.tensor_tensor(
                        tmp_t[m][:], f_t[m][:],
                        s_sb[:, m * BC:(m + 1) * BC], SUB)
                    # u = 0.1*tmp
                    nc.vector.tensor_scalar_mul(u_t[m][:], tmp_t[m][:], DT)
                if not last:
                    for m in range(MT):
                        # s += u.  Last delta round: DVE add (the DMA-add
                        # completion latency would gate the output chain).
                        if k < UNFOLDS - 2:
                            nc.gpsimd.dma_start(s_sb[:, m * BC:(m + 1) * BC],
                                                u_t[m][:], accum_op=ADD)
                        else:
                            nc.vector.tensor_tensor(
                                s_sb[:, m * BC:(m + 1) * BC],
                                s_sb[:, m * BC:(m + 1) * BC], u_t[m][:], ADD)
                    # psum += tmp @ (0.1*Wb).  Hybrid order: j0/j1 across all
                    # m, then (j2,j3) per m — ps[0] completes at 56% of the
                    # round so the next round's tanh+tmp path fully hides.
                    order = ([(j, m) for j in (0, 1) for m in range(MT)] +
                             [(j, m) for m in range(MT) for j in (2, 3)])
                    for j, m in order:
                        for c in range(NCH):
                            nc.tensor.matmul(
                                ps[m][:, c * CHUNK:(c + 1) * CHUNK],
                                lhsT=wslice(wd, j, m),
                                rhs=tmp_t[j][:, c * CHUNK:(c + 1) * CHUNK],
                                start=False,
                                stop=(k == UNFOLDS - 2 and j == 3),
                                skip_group_check=True,
                            )
                else:
                    o_t = [opool.tile([P, BC], f16, tag=f"o{m}",
                                      name=f"o{m}") for m in range(MT)]
                    for m in range(MT):
                        # out = s + u, all-DVE (lowest-latency exit path)
                        nc.vector.tensor_tensor(
                            o_t[m][:], s_sb[:, m * BC:(m + 1) * BC],
                            u_t[m][:], ADD)
                        out_eng = (nc.sync, nc.scalar, nc.sync, nc.scalar)[m]
                        out_eng.dma_start(outT[m * P:(m + 1) * P, :], o_t[m][:])

    nc.compile()
    return nc


def _get_nc():
    global _compiled_nc
    if _compiled_nc is None:
        _compiled_nc = _build_nc()
    return _compiled_nc


def make_in_maps(x, s, W, b):
    """Shard + pack host-side: x/s transposed to (128, k*1024) fp16 with
    k-tiles side by side; W packed to (128, 8*512) fp16 k-tile-major."""
    xT = np.ascontiguousarray(x.T)   # (D, B)
    sT = np.ascontiguousarray(s.T)   # (N, B)
    Wp = np.ascontiguousarray(
        W.reshape(8, P, N).transpose(1, 0, 2).reshape(P, -1)
    ).astype(np.float16)
    in_maps = []
    for c in range(NCORES):
        sl = slice(c * BC, (c + 1) * BC)
        xs = xT[:, sl].reshape(KT_X, P, BC).transpose(1, 0, 2).reshape(P, -1)
        ss = sT[:, sl].reshape(KT_S, P, BC).transpose(1, 0, 2).reshape(P, -1)
        in_maps.append({
            "xP": np.ascontiguousarray(xs).astype(np.float16),
            "sP": np.ascontiguousarray(ss).astype(np.float16),
            "Wp": Wp,
            "bias": b,
        })
    return in_maps


def kernel(**inputs):
    from concourse.bass_utils import run_bass_kernel_spmd

    x = np.asarray(inputs["inputs"], dtype=np.float32)
    s = np.asarray(inputs["state"], dtype=np.float32)
    W = np.ascontiguousarray(np.asarray(inputs["W"], dtype=np.float32))
    b = np.ascontiguousarray(np.asarray(inputs["bias"], dtype=np.float32))

    in_maps = make_in_maps(x, s, W, b)
    nc = _get_nc()
    res = run_bass_kernel_spmd(nc, in_maps, list(range(NCORES))).results
    outT = np.concatenate([res[c]["outT"] for c in range(NCORES)], axis=1)
    out = np.ascontiguousarray(outT.T).astype(np.float32)
    return (out, out)
